# revision 2
# baseline (speedup 1.0000x reference)
"""Trainium2 Bass kernel for nn_KSpaceLoss: exact type-2 NUFFT k-space loss.

loss = 0.1 * (sum|d| / sum|a|) + 0.1 * sqrt(sum d^2 / sum a^2)
  d = (E @ x) * mask - kdata * mask,  a = kdata * mask
  E[k, n] = exp(-2j*pi * traj[:, k] . r[:, n])   (K=8192, N=96*96)

Sharding: K axis split across 8 NeuronCores (1024 samples each). Each core:
  - PE:  phase[n, k] = gx[n]*tx[k] + gy[n]*ty[k]     (fp32 matmul, contraction=2)
  - VE:  m = mod(phase, 1); t = |m - 0.5|            (range reduction)
  - ACT: Er = sin(2pi*t - pi/2) = cos(2pi*phase); Ei = sin(2pi*m - pi) = -sin(..)
  - PE:  ksp[cst, k] += E^T-chunks @ x-weights       (bf16, accumulate over n)
  - VE/ACT: masked residual, |d|, d^2, |a|, a^2 partial sums -> [32, 4]
Host: final 4-scalar psum across cores/partitions + weighted combine.
"""

import math

import numpy as np
import ml_dtypes

import concourse.bacc as bacc
import concourse.tile as tile
from concourse import mybir
from concourse.bass_utils import run_bass_kernel_spmd

X, Y, Z = 96, 96, 1
C, S, T = 8, 1, 4
K = 8192
N = X * Y * Z
NCORES = 8
KL = K // NCORES          # 1024 k-samples per core
NCH = N // 128            # 72 chunks of 128 grid points
CST = C * S * T           # 32
W1, W2 = 0.1, 0.1

F32 = mybir.dt.float32
F32R = mybir.dt.float32r
U32 = mybir.dt.uint32
U16 = mybir.dt.uint16
F16 = mybir.dt.float16
BF16 = mybir.dt.bfloat16
PI = math.pi
MAGIC = 12582912.0          # 1.5 * 2^23: fl(x + MAGIC) - MAGIC == round(x)


def build_kernel():
    nc = bacc.Bacc("TRN2", target_bir_lowering=False, debug=False,
                   num_devices=NCORES)

    wr_d = nc.dram_tensor("wr", [128, NCH, CST], BF16, kind="ExternalInput").ap()
    wi_d = nc.dram_tensor("wi", [128, NCH, CST], BF16, kind="ExternalInput").ap()
    wm_d = nc.dram_tensor("wm", [128, NCH, CST], BF16, kind="ExternalInput").ap()
    r2_d = nc.dram_tensor("r2", [4, N], BF16, kind="ExternalInput").ap()
    tw_d = nc.dram_tensor("tw", [4, KL], BF16, kind="ExternalInput").ap()
    kdr_d = nc.dram_tensor("kdr", [CST, KL], F32, kind="ExternalInput").ap()
    kdi_d = nc.dram_tensor("kdi", [CST, KL], F32, kind="ExternalInput").ap()
    mkb_d = nc.dram_tensor("mkb", [CST, KL], F32, kind="ExternalInput").ap()
    parts_d = nc.dram_tensor("parts", [CST, 4], F32, kind="ExternalOutput").ap()

    Sin = mybir.ActivationFunctionType.Sin
    Sqrt = mybir.ActivationFunctionType.Sqrt
    Ident = mybir.ActivationFunctionType.Identity
    Alu = mybir.AluOpType

    with tile.TileContext(nc) as tc:
        with (
            tc.tile_pool(name="const", bufs=1) as cpool,
            tc.tile_pool(name="phase", bufs=2, space="PSUM") as php,
            tc.tile_pool(name="acc", bufs=1, space="PSUM") as accp,
            tc.tile_pool(name="work", bufs=3) as wkp,
            tc.tile_pool(name="ework", bufs=3) as ewp,
            tc.tile_pool(name="resid", bufs=1) as rsp,
        ):
            # constant loads
            wr = cpool.tile([128, NCH, CST], BF16, tag="wr")
            wi = cpool.tile([128, NCH, CST], BF16, tag="wi")
            wm = cpool.tile([128, NCH, CST], BF16, tag="wm")
            r2 = cpool.tile([4, N], BF16, tag="r2")
            tw = cpool.tile([4, KL], BF16, tag="tw")
            kdr = cpool.tile([CST, KL], F32, tag="kdr")
            kdi = cpool.tile([CST, KL], F32, tag="kdi")
            mkb = cpool.tile([CST, KL], F32, tag="mkb")
            nc.sync.dma_start(wr[:], wr_d[:])
            nc.sync.dma_start(wi[:], wi_d[:])
            nc.sync.dma_start(wm[:], wm_d[:])
            nc.sync.dma_start(r2[:], r2_d[:])
            nc.sync.dma_start(tw[:], tw_d[:])
            nc.sync.dma_start(kdr[:], kdr_d[:])
            nc.sync.dma_start(kdi[:], kdi_d[:])
            nc.sync.dma_start(mkb[:], mkb_d[:])

            bias_cos = cpool.tile([128, 1], F32, tag="bcos")
            nc.vector.memset(bias_cos[:], PI / 2)
            bias_magic = cpool.tile([128, 1], F32, tag="bmag")
            nc.vector.memset(bias_magic[:], MAGIC)

            ps_re = accp.tile([CST, KL], F32, tag="ps_re")
            ps_im = accp.tile([CST, KL], F32, tag="ps_im")

            NJ = KL // 512
            SC = 1                       # n-chunks per super-chunk
            SW = SC * KL                 # super-tile width
            for s in range(NCH // SC):
                ph = php.tile([128, SW], F32, tag="ph")
                for h in range(SC):
                    c = s * SC + h
                    lhsT = r2[:, c * 128:(c + 1) * 128]
                    for j in range(NJ):
                        sl = slice(h * KL + j * 512, h * KL + (j + 1) * 512)
                        nc.tensor.matmul(ph[:, sl], lhsT, tw[:, j * 512:(j + 1) * 512],
                                         start=True, stop=True)
                # range reduction: rnd' = MAGIC + round(ph) (VE/ACT alternating),
                # mf = (rnd' - MAGIC) - ph = -frac(ph) in fp16; u = |mf|
                rnd = wkp.tile([128, SW], F32, tag="rnd")
                if s % 2 == 0:
                    nc.vector.tensor_scalar(rnd[:], ph[:], MAGIC, None,
                                            op0=Alu.add)
                else:
                    nc.scalar.activation(rnd[:], ph[:], Ident,
                                         bias=bias_magic[:], scale=1.0)
                mf = wkp.tile([128, SW], F16, tag="mf")
                nc.vector.scalar_tensor_tensor(mf[:], rnd[:], MAGIC, ph[:],
                                               op0=Alu.subtract,
                                               op1=Alu.subtract)
                uu = wkp.tile([128, SW], F16, tag="uu")
                nc.vector.tensor_scalar(uu[:].bitcast(U16), mf[:].bitcast(U16),
                                        0x7FFF, None, op0=Alu.bitwise_and)
                # Er = cos(2pi*ph) = sin(pi/2 - 2pi*u); Ei = -sin(2pi*ph) = sin(2pi*mf)
                er = ewp.tile([128, SW], BF16, tag="er")
                nc.scalar.activation(er[:], uu[:], Sin,
                                     bias=bias_cos[:], scale=-2 * PI)
                ei = ewp.tile([128, SW], BF16, tag="ei")
                nc.scalar.activation(ei[:], mf[:], Sin, bias=0.0, scale=2 * PI)

                for h in range(SC):
                    c = s * SC + h
                    first = c == 0
                    last = c == NCH - 1
                    xr_s = wr[:, c, :]
                    xi_s = wi[:, c, :]
                    xm_s = wm[:, c, :]
                    off = h * KL
                    # grouped by stationary weight to maximize LDW reuse
                    for j in range(NJ):
                        sl = slice(off + j * 512, off + (j + 1) * 512)
                        nc.tensor.matmul(ps_re[:, j * 512:(j + 1) * 512], xr_s,
                                         er[:, sl], start=first, stop=False)
                    for j in range(NJ):
                        sl = slice(off + j * 512, off + (j + 1) * 512)
                        nc.tensor.matmul(ps_im[:, j * 512:(j + 1) * 512], xr_s,
                                         ei[:, sl], start=first, stop=False)
                    for j in range(NJ):
                        sl = slice(off + j * 512, off + (j + 1) * 512)
                        nc.tensor.matmul(ps_re[:, j * 512:(j + 1) * 512], xm_s,
                                         ei[:, sl], start=False, stop=last)
                    for j in range(NJ):
                        sl = slice(off + j * 512, off + (j + 1) * 512)
                        nc.tensor.matmul(ps_im[:, j * 512:(j + 1) * 512], xi_s,
                                         er[:, sl], start=False, stop=last)

            # residual: d = ksp*mask - kdm ; partial sums over k per partition
            parts = rsp.tile([CST, 4], F32, tag="parts")
            dm_r = rsp.tile([CST, KL], F32, tag="dm_r")
            dm_i = rsp.tile([CST, KL], F32, tag="dm_i")
            sq = rsp.tile([CST, KL], F32, tag="sq")
            sq2 = rsp.tile([CST, KL], F32, tag="sq2")
            p1 = rsp.tile([CST, 1], F32, tag="p1")
            p2 = rsp.tile([CST, 1], F32, tag="p2")

            # d (masked): (psum * mask) - kdm
            nc.vector.scalar_tensor_tensor(dm_r[:], ps_re[:], 0.0, mkb[:],
                                           op0=Alu.bypass, op1=Alu.mult)
            nc.vector.tensor_tensor(dm_r[:], dm_r[:], kdr[:], op=Alu.subtract)
            nc.vector.scalar_tensor_tensor(dm_i[:], ps_im[:], 0.0, mkb[:],
                                           op0=Alu.bypass, op1=Alu.mult)
            nc.vector.tensor_tensor(dm_i[:], dm_i[:], kdi[:], op=Alu.subtract)
            # |d|^2 partial sums
            nc.vector.scalar_tensor_tensor(sq[:], dm_r[:], 0.0, dm_r[:],
                                           op0=Alu.bypass, op1=Alu.mult,
                                           accum_out=p1[:])
            nc.vector.scalar_tensor_tensor(sq2[:], dm_i[:], 0.0, dm_i[:],
                                           op0=Alu.bypass, op1=Alu.mult,
                                           accum_out=p2[:])
            nc.vector.tensor_tensor(parts[:, 1:2], p1[:], p2[:], op=Alu.add)
            nc.vector.tensor_tensor(sq[:], sq[:], sq2[:], op=Alu.add)
            nc.scalar.activation(dm_r[:], sq[:], Sqrt, accum_out=parts[:, 0:1])
            # |a|^2 partial sums (kdm is pre-masked on host)
            nc.vector.scalar_tensor_tensor(sq[:], kdr[:], 0.0, kdr[:],
                                           op0=Alu.bypass, op1=Alu.mult,
                                           accum_out=p1[:])
            nc.vector.scalar_tensor_tensor(sq2[:], kdi[:], 0.0, kdi[:],
                                           op0=Alu.bypass, op1=Alu.mult,
                                           accum_out=p2[:])
            nc.vector.tensor_tensor(parts[:, 3:4], p1[:], p2[:], op=Alu.add)
            nc.vector.tensor_tensor(sq[:], sq[:], sq2[:], op=Alu.add)
            nc.scalar.activation(dm_i[:], sq[:], Sqrt, accum_out=parts[:, 2:3])

            nc.sync.dma_start(parts_d[:], parts[:])

    nc.compile()
    return nc


_NC_CACHE = []


def _get_nc():
    if not _NC_CACHE:
        _NC_CACHE.append(build_kernel())
    return _NC_CACHE[0]


def make_in_maps(images_reconstructed, kspace_trajectory, kspace_data,
                 kspace_mask, sensitivity_maps):
    img = np.asarray(images_reconstructed)
    traj = np.asarray(kspace_trajectory).astype(np.float32)
    kdata = np.asarray(kspace_data)
    mask = np.asarray(kspace_mask).astype(np.float32)
    smaps = np.asarray(sensitivity_maps)

    x = 0.5 * img[None, ...] * smaps[..., None, None]      # (C,X,Y,Z,S,T)
    xw = x.reshape(C, N, T).transpose(1, 0, 2).reshape(N, CST)
    bf = ml_dtypes.bfloat16
    wr = np.ascontiguousarray(
        xw.real.astype(bf).reshape(NCH, 128, CST).transpose(1, 0, 2))
    wi = np.ascontiguousarray(
        xw.imag.astype(bf).reshape(NCH, 128, CST).transpose(1, 0, 2))
    wm = np.ascontiguousarray(
        (-xw.imag).astype(bf).reshape(NCH, 128, CST).transpose(1, 0, 2))

    gx = np.arange(X, dtype=np.float32) - X // 2
    gy = np.arange(Y, dtype=np.float32) - Y // 2
    rx, ry = np.repeat(gx, Y), np.tile(gy, X)
    # split-precision phase operands: grid coords are exact in bf16;
    # traj rows split hi/lo so bf16 matmul reproduces fp32 phase to ~1e-4
    r2 = np.stack([rx, rx, ry, ry]).astype(bf)
    t2 = traj[:2]
    th = t2.astype(bf)
    tl = (t2 - th.astype(np.float32)).astype(bf)
    tw4 = np.stack([th[0], tl[0], th[1], tl[1]])

    kdm = (kdata * mask).reshape(C, K, T).transpose(1, 0, 2).reshape(K, CST)
    mk = mask.reshape(K).astype(np.float32)

    in_maps = []
    for i in range(NCORES):
        ksl = slice(i * KL, (i + 1) * KL)
        in_maps.append({
            "wr": wr, "wi": wi, "wm": wm, "r2": r2,
            "tw": np.ascontiguousarray(tw4[:, ksl]),
            "kdr": np.ascontiguousarray(kdm.real[ksl].T.astype(np.float32)),
            "kdi": np.ascontiguousarray(kdm.imag[ksl].T.astype(np.float32)),
            "mkb": np.ascontiguousarray(
                np.broadcast_to(mk[ksl][None, :], (CST, KL))),
        })
    return in_maps


def combine(parts_list):
    tot = np.zeros(4, dtype=np.float64)
    for p in parts_list:
        tot += p.astype(np.float64).sum(axis=0)
    loss = W1 * (tot[0] / tot[2]) + W2 * math.sqrt(tot[1] / tot[3])
    return np.asarray(loss, dtype=np.float32)


def kernel(images_reconstructed, kspace_trajectory, kspace_data,
           kspace_mask, sensitivity_maps, _trace=False):
    nc = _get_nc()
    in_maps = make_in_maps(images_reconstructed, kspace_trajectory,
                           kspace_data, kspace_mask, sensitivity_maps)
    kw = {"tmpdir": "/tmp/bass_trace"} if _trace else {}
    res = run_bass_kernel_spmd(nc, in_maps, core_ids=list(range(NCORES)),
                               trace=_trace, **kw)
    out = combine([res.results[i]["parts"] for i in range(NCORES)])
    if _trace:
        return out, res
    return out



# revision 5
# speedup vs baseline: 1.7415x; 1.7415x over previous
"""Trainium2 Bass kernel for nn_KSpaceLoss: exact type-2 NUFFT k-space loss.

loss = 0.1 * (sum|d| / sum|a|) + 0.1 * sqrt(sum d^2 / sum a^2)
  d = (E @ x) * mask - kdata * mask,  a = kdata * mask
  E[k, n] = exp(-2j*pi * traj[:, k] . r[:, n])   (K=8192, N=96*96)

Sharding: K axis split across 8 NeuronCores (1024 samples each).

v2 structure (per core):
 - Mirror pairing: E(-r) = conj(E(r)); host pairs grid points r/-r, so only
   4704 representative points (38 chunks of 128, padded) need phase/trig.
   Paired contribution with u± = xr ± xr', v± = xi ± xi':
     Re += er*u+ + ei*(-v-) ;  Im += er*v+ + ei*u-
 - PE: ph = 4-row split-precision phase matmul (fp32 PSUM)
       rnd = 5-row matmul with MAGIC row last -> fl(phase+MAGIC) exactly
 - VE: mf = (rnd - MAGIC) - ph = -frac(phase)  (f16), uu = |mf| (bitand)
 - ACT: er = sin(pi/2 - 2pi*uu) = cos(2pi*ph); ei = sin(2pi*mf) = -sin(2pi*ph)
 - PE: ps[0:64, k] accumulates er-stream @ [u+;v+] and ei-stream @ [-v-;u-]
 - residual: d = ps*mask - kdata*mask on (64,k); L2 partials via accum;
   |d| via fp32 pairing matmul (dr^2+di^2) + sqrt-accum. a-side sums on host.
"""

import math

import numpy as np
import ml_dtypes

import concourse.bacc as bacc
import concourse.tile as tile
from concourse import mybir
from concourse.bass_utils import run_bass_kernel_spmd

X, Y, Z = 96, 96, 1
C, S, T = 8, 1, 4
K = 8192
N = X * Y * Z
NCORES = 8
KL = K // NCORES          # 1024 k-samples per core
NR = 4864                 # padded representative points (38 chunks)
NCH = NR // 128           # 38
SC = 2                    # chunks per sin supertile
CST = C * S * T           # 32
W1, W2 = 0.1, 0.1

F32 = mybir.dt.float32
U16 = mybir.dt.uint16
F16 = mybir.dt.float16
BF16 = mybir.dt.bfloat16
PI = math.pi
MAGIC = 12582912.0          # 1.5 * 2^23: fl(x + MAGIC) - MAGIC == round(x)


def build_kernel():
    nc = bacc.Bacc("TRN2", target_bir_lowering=False, debug=False,
                   num_devices=NCORES)

    w1_d = nc.dram_tensor("w1", [128, NCH, 64], BF16, kind="ExternalInput").ap()
    w2_d = nc.dram_tensor("w2", [128, NCH, 64], BF16, kind="ExternalInput").ap()
    r2_d = nc.dram_tensor("r2", [4, NR], BF16, kind="ExternalInput").ap()
    tw_d = nc.dram_tensor("tw", [4, KL], BF16, kind="ExternalInput").ap()
    kdm_d = nc.dram_tensor("kdm", [64, KL], F32, kind="ExternalInput").ap()
    mkb_d = nc.dram_tensor("mkb", [64, KL], F32, kind="ExternalInput").ap()
    pm_d = nc.dram_tensor("pm", [64, 32], F32, kind="ExternalInput").ap()
    pl1_d = nc.dram_tensor("pl1", [32, 1], F32, kind="ExternalOutput").ap()
    pl2_d = nc.dram_tensor("pl2", [64, 1], F32, kind="ExternalOutput").ap()

    Sin = mybir.ActivationFunctionType.Sin
    Sqrt = mybir.ActivationFunctionType.Sqrt
    Ident = mybir.ActivationFunctionType.Identity
    Alu = mybir.AluOpType

    with tile.TileContext(nc) as tc:
        with (
            tc.tile_pool(name="const", bufs=1) as cpool,
            tc.tile_pool(name="ph", bufs=2, space="PSUM") as php,
            tc.tile_pool(name="acc", bufs=1, space="PSUM") as accp,
            tc.tile_pool(name="rnd", bufs=2) as rnp,
            tc.tile_pool(name="mwork", bufs=2) as vwp,
            tc.tile_pool(name="ework", bufs=2) as ewp,
            tc.tile_pool(name="resid", bufs=1) as rsp,
        ):
            r2 = cpool.tile([4, NR], BF16, tag="r2")
            tw = cpool.tile([4, KL], BF16, tag="tw")
            w1 = cpool.tile([128, NCH, 64], BF16, tag="w1")
            w2 = cpool.tile([128, NCH, 64], BF16, tag="w2")
            kdm = cpool.tile([64, KL], F32, tag="kdm")
            mkb = cpool.tile([64, KL], F32, tag="mkb")
            pm = cpool.tile([64, 32], F32, tag="pm")
            nc.sync.dma_start(r2[:], r2_d[:])
            nc.sync.dma_start(tw[:], tw_d[:])
            nc.sync.dma_start(w1[:], w1_d[:])
            nc.sync.dma_start(w2[:], w2_d[:])
            nc.sync.dma_start(kdm[:], kdm_d[:])
            nc.sync.dma_start(mkb[:], mkb_d[:])
            nc.sync.dma_start(pm[:], pm_d[:])

            bias_cos = cpool.tile([128, 1], F32, tag="bcos")
            nc.vector.memset(bias_cos[:], PI / 2)
            bias_magic = cpool.tile([128, 1], F32, tag="bmag")
            nc.vector.memset(bias_magic[:], MAGIC)

            ps = accp.tile([64, KL], F32, tag="ps")

            for s in range(NCH // SC):
                mfT = vwp.tile([128, SC, KL], F16, tag="mf")
                uuT = vwp.tile([128, SC, KL], F16, tag="uu")
                erT = ewp.tile([128, SC, KL], BF16, tag="er")
                eiT = ewp.tile([128, SC, KL], BF16, tag="ei")
                for h in range(SC):
                    c = s * SC + h
                    lhs4 = r2[0:4, c * 128:(c + 1) * 128]
                    ph = php.tile([128, KL], F32, tag="ph")
                    for q in range(2):
                        sl = slice(q * 512, (q + 1) * 512)
                        nc.tensor.matmul(ph[:, sl], lhs4, tw[0:4, sl],
                                         start=True, stop=True)
                    rndS = rnp.tile([128, KL], F32, tag="rnd")
                    if c % 7 < 4:
                        nc.scalar.activation(rndS[:], ph[:], Ident,
                                             bias=bias_magic[:], scale=1.0)
                    else:
                        nc.vector.tensor_scalar(rndS[:], ph[:], MAGIC, None,
                                                op0=Alu.add)
                    nc.vector.scalar_tensor_tensor(
                        mfT[:, h, :], rndS[:], MAGIC, ph[:],
                        op0=Alu.subtract, op1=Alu.subtract)
                    nc.vector.tensor_scalar(
                        uuT[:, h, :].bitcast(U16), mfT[:, h, :].bitcast(U16),
                        0x7FFF, None, op0=Alu.bitwise_and)
                nc.scalar.activation(erT[:], uuT[:], Sin,
                                     bias=bias_cos[:], scale=-2 * PI)
                nc.scalar.activation(eiT[:], mfT[:], Sin, bias=0.0,
                                     scale=2 * PI)
                for h in range(SC):
                    c = s * SC + h
                    first = c == 0
                    last = c == NCH - 1
                    for j in range(2):
                        sl = slice(j * 512, (j + 1) * 512)
                        nc.tensor.matmul(ps[:, sl], w1[:, c, :], erT[:, h, sl],
                                         start=first, stop=False)
                    for j in range(2):
                        sl = slice(j * 512, (j + 1) * 512)
                        nc.tensor.matmul(ps[:, sl], w2[:, c, :], eiT[:, h, sl],
                                         start=False, stop=last)

            # residual: d = ps*mask - kdm on (64, KL); partial sums
            dm = rsp.tile([64, KL], F32, tag="dm")
            dd = rsp.tile([64, KL], F32, tag="dd")
            sink = rsp.tile([32, KL], BF16, tag="sink")
            pl1 = rsp.tile([32, 1], F32, tag="pl1")
            pl2 = rsp.tile([64, 1], F32, tag="pl2")
            ss = accp.tile([32, KL], F32, tag="ss")

            nc.vector.scalar_tensor_tensor(dm[:], ps[:], 0.0, mkb[:],
                                           op0=Alu.bypass, op1=Alu.mult)
            nc.vector.tensor_tensor(dm[:], dm[:], kdm[:], op=Alu.subtract)
            nc.vector.scalar_tensor_tensor(dd[:], dm[:], 0.0, dm[:],
                                           op0=Alu.bypass, op1=Alu.mult,
                                           accum_out=pl2[:])
            for j in range(2):
                sl = slice(j * 512, (j + 1) * 512)
                nc.tensor.matmul(ss[:, sl], pm[:], dd[:, sl],
                                 start=True, stop=True)
            nc.scalar.activation(sink[:], ss[:], Sqrt, accum_out=pl1[:])

            nc.sync.dma_start(pl1_d[:], pl1[:])
            nc.sync.dma_start(pl2_d[:], pl2[:])

    nc.compile()
    return nc


_NC_CACHE = []


def _get_nc():
    if not _NC_CACHE:
        _NC_CACHE.append(build_kernel())
    return _NC_CACHE[0]


def _host_prep(images_reconstructed, kspace_trajectory, kspace_data,
               kspace_mask, sensitivity_maps):
    img = np.asarray(images_reconstructed)
    traj = np.asarray(kspace_trajectory).astype(np.float32)
    kdata = np.asarray(kspace_data)
    mask = np.asarray(kspace_mask).astype(np.float32)
    smaps = np.asarray(sensitivity_maps)
    bf = ml_dtypes.bfloat16

    x = 0.5 * img[None, ...] * smaps[..., None, None]      # (C,X,Y,Z,S,T)
    xw = x.reshape(C, N, T).transpose(1, 0, 2).reshape(N, CST)

    # mirror pairing: E(-r) = conj(E(r))
    GX, GY = np.meshgrid(np.arange(X) - 48, np.arange(Y) - 48, indexing="ij")
    gxf, gyf = GX.ravel(), GY.ravel()
    n_arr = np.arange(N)
    has_m = (gxf >= -47) & (gyf >= -47)
    mirror_n = np.where(has_m, (48 - gxf) * 96 + (48 - gyf), -1)
    is_rep = (~has_m) | (n_arr <= mirror_n)
    idx = n_arr[is_rep]
    midx = mirror_n[is_rep]
    midx = np.where(midx == idx, -1, midx)
    pad = NR - len(idx)

    xr = xw.real.astype(np.float32)
    xi = xw.imag.astype(np.float32)
    sel = np.maximum(midx, 0)
    on = (midx[:, None] >= 0)
    xr_m = np.where(on, xr[sel], 0.0)
    xi_m = np.where(on, xi[sel], 0.0)
    w1 = np.concatenate([xr[idx] + xr_m, xi[idx] + xi_m], 1)   # [u+; v+]
    w2 = np.concatenate([-(xi[idx] - xi_m), xr[idx] - xr_m], 1)  # [-v-; u-]
    zpad = np.zeros((pad, 64), np.float32)
    w1 = np.ascontiguousarray(np.vstack([w1, zpad]).astype(bf)
                              .reshape(NCH, 128, 64).transpose(1, 0, 2))
    w2 = np.ascontiguousarray(np.vstack([w2, zpad]).astype(bf)
                              .reshape(NCH, 128, 64).transpose(1, 0, 2))

    gxr = np.concatenate([gxf[is_rep], np.zeros(pad)]).astype(np.float32)
    gyr = np.concatenate([gyf[is_rep], np.zeros(pad)]).astype(np.float32)
    r2 = np.stack([gxr, gxr, gyr, gyr]).astype(bf)

    t2 = traj[:2]
    th = t2.astype(bf)
    tl = (t2 - th.astype(np.float32)).astype(bf)
    tw5 = np.stack([th[0], tl[0], th[1], tl[1]])

    mk = mask.reshape(K).astype(np.float32)
    kd = kdata.reshape(C, K, T).transpose(1, 0, 2).reshape(K, CST)
    kdm = kd * mk[:, None]
    kdm64 = np.concatenate([kdm.real, kdm.imag], 1).T.astype(np.float32)  # (64,K)
    mkb64 = np.broadcast_to(mk[None, :], (64, K)).astype(np.float32)

    pm = np.zeros((64, 32), np.float32)
    pm[np.arange(64), np.arange(64) % 32] = 1.0

    # a-side norms on host (fp64)
    a1 = np.abs(kdm).astype(np.float64).sum()
    a2 = (np.abs(kdm).astype(np.float64) ** 2).sum()

    in_maps = []
    for i in range(NCORES):
        ksl = slice(i * KL, (i + 1) * KL)
        in_maps.append({
            "w1": w1, "w2": w2, "r2": r2,
            "tw": np.ascontiguousarray(tw5[:, ksl]),
            "kdm": np.ascontiguousarray(kdm64[:, ksl]),
            "mkb": np.ascontiguousarray(mkb64[:, ksl]),
            "pm": pm,
        })
    return in_maps, a1, a2


def kernel(images_reconstructed, kspace_trajectory, kspace_data,
           kspace_mask, sensitivity_maps, _trace=False):
    nc = _get_nc()
    in_maps, a1, a2 = _host_prep(images_reconstructed, kspace_trajectory,
                                 kspace_data, kspace_mask, sensitivity_maps)
    kw = {"tmpdir": "/tmp/bass_trace"} if _trace else {}
    res = run_bass_kernel_spmd(nc, in_maps, core_ids=list(range(NCORES)),
                               trace=_trace, **kw)
    l1 = sum(res.results[i]["pl1"].astype(np.float64).sum()
             for i in range(NCORES))
    l2 = sum(res.results[i]["pl2"].astype(np.float64).sum()
             for i in range(NCORES))
    loss = np.asarray(W1 * (l1 / a1) + W2 * math.sqrt(l2) / math.sqrt(a2),
                      dtype=np.float32)
    if _trace:
        return loss, res
    return loss
